# revision 29
# baseline (speedup 1.0000x reference)
"""Trainium2 Bass kernel: NKQuantizer2 top-k masking (k=8).

reference:  kh = topk_hot(x, 8)          # [B,S,Q] 0/1 mask, top-8 per token
            out = einsum('bsq,eq->bse', kh, W)

Per token: out[t] = sum_{q in top8(x[t])} W[:, q] -- an 8-way embedding
gather-sum from W.T [Q, E].

Strategy (data-parallel over tokens across 8 cores, W.T bf16 in HBM):
  Per 128-token tile on each core:
    1. DMA x tile [128, 8192] f32 HBM->SBUF on the SP HWDGE ring
    2. DVE Max8 -> top-8 values per token; DVE MaxIndex -> their indices
       (exact, ties -> first occurrence, matching jax.lax.top_k)
    3. 8 INDEPENDENT single-index indirect gathers (bf16, bypass) into a
       wide [128, 8, 512] tile -- no CCE accumulate chain, so the DMAs
       carry no waits and flow at Q7 descriptor-gen cadence (~1.9us).
    4. DVE 3-step tree-reduce (bf16) collapses the 8 rows; a SWDGE store
       casts bf16->f32 to DRAM.

Why no CCE accumulation: a CCE add chain needs each link to wait its
predecessor's DMA *completion* (~4.6us/link serial per tile, and
sem-free accumulation races in the SDMA datapath -- measured).  The
independent-gather design has zero inter-DMA dependencies; the extra
~2.3us/tile of DVE reduce rides on the DVE bottleneck (~17.4 ->
~19.7us/tile) but removes all FIFO head-of-line stalls.

Toolchain constraints handled:
  - at most ONE semaphore wait per instruction: every content wait
    rides a dedicated Pool NOP (the Pool queue is serial, so queue
    order gates the dep-free DMAs); the DVE reduce is gated by one
    sync dep on the last of 8 lane-observation NOPs (whose Pool-queue
    program order observed all 8 gather completions).
  - all Pool-queue instructions are nosync-chained in emission order
    so the scheduler cannot reorder the queue.
"""

import numpy as np
import ml_dtypes

import concourse.bass as bass
import concourse.mybir as mybir
import concourse.tile as tile
from concourse.bass_utils import run_bass_kernel_spmd
from concourse.tile_rust import add_dep_helper

B, S, Q, E, TOPK = 4, 2048, 8192, 512, 8
N_CORES = 8
P = 128
T_TOTAL = B * S                 # 8192 tokens
T_CORE = T_TOTAL // N_CORES     # 1024 tokens per core

F32 = mybir.dt.float32
BF16 = mybir.dt.bfloat16
U32 = mybir.dt.uint32


def build_bass(t_core=T_CORE, q=Q, e=E):
    """Build the per-core Bass program (SPMD: same program on all cores)."""
    n_tiles = t_core // P
    assert n_tiles == 8
    xbufs = 4

    nc = bass.Bass(trn_type="TRN2", target_bir_lowering=False)
    x_d = nc.dram_tensor("x", [t_core, q], F32, kind="ExternalInput")
    wt_d = nc.dram_tensor("wt", [q, e], BF16, kind="ExternalInput")
    out_d = nc.dram_tensor("out", [t_core, e], F32, kind="ExternalOutput")

    pool_q = []  # ALL Pool-queue instructions (DMAs + NOPs) in queue order

    def strip_sync_deps(bi):
        try:
            names = list(bi.ins.sync_dependency_names())
        except TypeError:
            names = list(bi.ins.sync_dependency_names)
        for n in names:
            bi.ins.try_remove_dependency(n)

    def poolq(bi):
        """nosync-chain every Pool instruction in emission order."""
        if pool_q:
            add_dep_helper(bi.ins, pool_q[-1].ins, False, "pool queue order")
        pool_q.append(bi)
        return bi

    with tile.TileContext(nc) as tc:
        with (
            tc.tile_pool(name="xpool", bufs=xbufs) as xpool,
            tc.tile_pool(name="spool", bufs=n_tiles) as spool,
            tc.tile_pool(name="ipool", bufs=n_tiles) as ipool,
            tc.tile_pool(name="gpool", bufs=n_tiles) as gpool,
        ):
            xts = [xpool.tile([P, q], F32, name="xt", tag="xt") for _ in range(xbufs)]
            idx8s, gws, i_idxs, xls = [], [], [], []
            gathers = {}     # (i, j) -> dma
            lane_nops = {}   # i -> last lane-observation nop
            red3 = {}        # i -> last reduce instr
            ostores = []

            def emit_xload(i):
                xt = xts[i % xbufs]
                dma = nc.sync.dma_start(xt[:], x_d[i * P : (i + 1) * P, :])
                if i >= xbufs:
                    add_dep_helper(
                        dma.ins, i_idxs[i - xbufs].ins, True, "xt WAR"
                    )
                    dma.ins.try_remove_dependency(xls[i - xbufs].ins.name)
                xls.append(dma)
                return dma

            def emit_topk(i):
                xt = xts[i % xbufs]
                s8 = spool.tile([P, 8], F32, name="s8", tag="s8")
                m8 = nc.vector.max(out=s8[:], in_=xt[:])
                if i_idxs:
                    # keep the DVE alternating m,f,m,f so find_index
                    # results come out as early as possible
                    add_dep_helper(m8.ins, i_idxs[-1].ins, False, "dve order")
                if i - 1 in red3:
                    add_dep_helper(m8.ins, red3[i - 1].ins, False, "dve order")
                idx8 = ipool.tile([P, 8], U32, name="idx8", tag="idx8")
                i_idx = nc.vector.max_index(
                    out=idx8[:], in_max=s8[:], in_values=xt[:]
                )
                idx8s.append(idx8)
                i_idxs.append(i_idx)
                gws.append(gpool.tile([P, TOPK, e], BF16, name="gw", tag="gw"))

            def emit_gathers(i):
                # gate: the Pool queue waits find_index8_i once, then all 8
                # dep-free gathers flow at Q7 cadence
                gate = poolq(nc.gpsimd.nop())
                add_dep_helper(gate.ins, i_idxs[i].ins, True, "idx ready")
                for j in range(TOPK):
                    dma = nc.gpsimd.indirect_dma_start(
                        out=gws[i][:, j, :],
                        out_offset=None,
                        in_=wt_d[:],
                        in_offset=bass.IndirectOffsetOnAxis(
                            ap=idx8s[i][:, j : j + 1], axis=0
                        ),
                        compute_op=mybir.AluOpType.bypass,
                    )
                    dma.ins.try_remove_dependency(i_idxs[i].ins.name)
                    poolq(dma)
                    gathers[(i, j)] = dma


            def emit_reduce(i):
                # 7 DVE nops each wait one gather's completion; DVE program
                # order accumulates their clocks, so a1 only needs the last
                # gather's wait itself -- one sem wait per instruction.
                # The whole reduce block is pinned AFTER the NEXT tile's
                # find_index8 so the gather latency stays out of the DVE
                # critical loop (max8/find_index must run back-to-back).
                gw = gws[i]
                first = None
                for j in range(TOPK - 1):
                    vn = nc.vector.nop()
                    strip_sync_deps(vn)
                    add_dep_helper(
                        vn.ins, gathers[(i, j)].ins, True, "gather done"
                    )
                    if first is None:
                        first = vn
                        if len(i_idxs) > i + 1:
                            add_dep_helper(
                                vn.ins, i_idxs[i + 1].ins, False, "dve order"
                            )
                a1 = nc.vector.tensor_add(
                    gw[:, 0:4, :], gw[:, 0:4, :], gw[:, 4:8, :]
                )
                strip_sync_deps(a1)
                add_dep_helper(
                    a1.ins, gathers[(i, TOPK - 1)].ins, True, "gathers done"
                )
                a2 = nc.vector.tensor_add(
                    gw[:, 0:2, :], gw[:, 0:2, :], gw[:, 2:4, :]
                )
                strip_sync_deps(a2)
                add_dep_helper(a2.ins, a1.ins, False, "dve order")
                a3 = nc.vector.tensor_add(
                    gw[:, 0, :], gw[:, 0, :], gw[:, 1, :]
                )
                strip_sync_deps(a3)
                add_dep_helper(a3.ins, a2.ins, False, "dve order")
                red3[i] = a3

            def emit_ostore(i):
                # SWDGE store with bf16 -> f32 cast; gated by a Pool NOP
                # carrying the DVE-reduce dependency
                gate = poolq(nc.gpsimd.nop())
                add_dep_helper(gate.ins, red3[i].ins, True, "reduce done")
                dma = nc.gpsimd.dma_start(
                    out_d[i * P : (i + 1) * P, :], gws[i][:, 0, :]
                )
                strip_sync_deps(dma)
                poolq(dma)
                ostores.append(dma)
                return dma

            for i in range(n_tiles):
                emit_xload(i)
                emit_topk(i)
                if i >= 1:
                    emit_reduce(i - 1)
                emit_gathers(i)
                if i >= 2:
                    emit_ostore(i - 2)
            emit_reduce(n_tiles - 1)
            emit_ostore(n_tiles - 2)
            emit_ostore(n_tiles - 1)

            # Quiesce procs with single-wait SP nops so the kernel-tail
            # drains find their required ticks already observed.
            tail = (
                xls
                + ostores
                + [gathers[(n_tiles - 1, j)] for j in range(TOPK)]
                + [red3[n_tiles - 1]]
                + i_idxs[-1:]
            )
            for tgt in tail:
                n = nc.sync.nop()
                add_dep_helper(n.ins, tgt.ins, True, "tail quiesce")

    return nc


def validate_single_wait(nc):
    """Every instruction may carry at most one semaphore wait."""
    bad = []
    for b in nc.m.functions[0].blocks:
        for ins in b.instructions:
            si = ins.sync_info
            if si is not None and len(si.on_wait) > 1:
                bad.append((ins.name, type(ins).__name__, si.on_wait))
    return bad


def _prep_wt(W: np.ndarray) -> np.ndarray:
    """W [e, q] f32 -> WT [q, e] bf16 contiguous."""
    return np.ascontiguousarray(W.T).astype(ml_dtypes.bfloat16)


_CACHED = {}


def _get_nc():
    if "nc" not in _CACHED:
        _CACHED["nc"] = build_bass()
    return _CACHED["nc"]


def kernel(x: np.ndarray, W: np.ndarray) -> np.ndarray:
    x = np.asarray(x, dtype=np.float32)
    W = np.asarray(W, dtype=np.float32)
    assert x.shape == (B, S, Q) and W.shape == (E, Q)

    nc = _get_nc()
    xf = x.reshape(T_TOTAL, Q)
    WT = _prep_wt(W)
    in_maps = [
        {
            "x": np.ascontiguousarray(xf[c * T_CORE : (c + 1) * T_CORE]),
            "wt": WT,
        }
        for c in range(N_CORES)
    ]
    res = run_bass_kernel_spmd(nc, in_maps, core_ids=list(range(N_CORES)))
    out = np.concatenate([r["out"] for r in res.results], axis=0)
    return np.ascontiguousarray(out.reshape(B, S, E).astype(np.float32))


# revision 30
# speedup vs baseline: 1.2072x; 1.2072x over previous
"""Trainium2 Bass kernel: NKQuantizer2 top-k masking (k=8).

reference:  kh = topk_hot(x, 8)          # [B,S,Q] 0/1 mask, top-8 per token
            out = einsum('bsq,eq->bse', kh, W)

Per token: out[t] = sum_{q in top8(x[t])} W[:, q] -- an 8-way embedding
gather-sum from W.T [Q, E].

Strategy (data-parallel over tokens across 8 cores, W.T bf16 in HBM):
  Per 128-token tile on each core:
    1. DMA x tile [128, 8192] f32 HBM->SBUF on the SP HWDGE ring
    2. DVE Max8 -> top-8 values per token; DVE MaxIndex -> their indices
       (exact, ties -> first occurrence, matching jax.lax.top_k)
    3. 8 INDEPENDENT single-index indirect gathers (bf16, bypass) into a
       wide [128, 8, 512] tile -- no CCE accumulate chain, so the DMAs
       carry no waits and flow at Q7 descriptor-gen cadence (~1.9us).
    4. DVE 3-step tree-reduce (bf16) collapses the 8 rows; a SWDGE store
       casts bf16->f32 to DRAM.

Why no CCE accumulation: a CCE add chain needs each link to wait its
predecessor's DMA *completion* (~4.6us/link serial per tile, and
sem-free accumulation races in the SDMA datapath -- measured).  The
independent-gather design has zero inter-DMA dependencies; the extra
~2.3us/tile of DVE reduce rides on the DVE bottleneck (~17.4 ->
~19.7us/tile) but removes all FIFO head-of-line stalls.

Toolchain constraints handled:
  - at most ONE semaphore wait per instruction: every content wait
    rides a dedicated Pool NOP (the Pool queue is serial, so queue
    order gates the dep-free DMAs); the DVE reduce is gated by one
    sync dep on the last of 8 lane-observation NOPs (whose Pool-queue
    program order observed all 8 gather completions).
  - all Pool-queue instructions are nosync-chained in emission order
    so the scheduler cannot reorder the queue.
"""

import numpy as np
import ml_dtypes

import concourse.bass as bass
import concourse.mybir as mybir
import concourse.tile as tile
from concourse.bass_utils import run_bass_kernel_spmd
from concourse.tile_rust import add_dep_helper

B, S, Q, E, TOPK = 4, 2048, 8192, 512, 8
N_CORES = 8
P = 128
T_TOTAL = B * S                 # 8192 tokens
T_CORE = T_TOTAL // N_CORES     # 1024 tokens per core

F32 = mybir.dt.float32
BF16 = mybir.dt.bfloat16
U32 = mybir.dt.uint32


def build_bass(t_core=T_CORE, q=Q, e=E):
    """Build the per-core Bass program (SPMD: same program on all cores)."""
    n_tiles = t_core // P
    assert n_tiles == 8
    xbufs = 4

    nc = bass.Bass(trn_type="TRN2", target_bir_lowering=False)
    x_d = nc.dram_tensor("x", [t_core, q], F32, kind="ExternalInput")
    wt_d = nc.dram_tensor("wt", [q, e], BF16, kind="ExternalInput")
    out_d = nc.dram_tensor("out", [t_core, e], F32, kind="ExternalOutput")

    pool_q = []  # ALL Pool-queue instructions (DMAs + NOPs) in queue order
    dve_q = []   # ALL DVE instructions in intended execution order

    def strip_sync_deps(bi):
        try:
            names = list(bi.ins.sync_dependency_names())
        except TypeError:
            names = list(bi.ins.sync_dependency_names)
        for n in names:
            bi.ins.try_remove_dependency(n)

    def poolq(bi):
        """nosync-chain every Pool instruction in emission order."""
        if pool_q:
            add_dep_helper(bi.ins, pool_q[-1].ins, False, "pool queue order")
        pool_q.append(bi)
        return bi

    def dveq(bi):
        """nosync-chain every DVE instruction in emission order."""
        if dve_q:
            add_dep_helper(bi.ins, dve_q[-1].ins, False, "dve queue order")
        dve_q.append(bi)
        return bi

    with tile.TileContext(nc) as tc:
        with (
            tc.tile_pool(name="xpool", bufs=xbufs) as xpool,
            tc.tile_pool(name="spool", bufs=n_tiles) as spool,
            tc.tile_pool(name="ipool", bufs=n_tiles) as ipool,
            tc.tile_pool(name="gpool", bufs=n_tiles) as gpool,
        ):
            xts = [xpool.tile([P, q], F32, name="xt", tag="xt") for _ in range(xbufs)]
            idx8s, gws, i_idxs, xls = [], [], [], []
            gathers = {}     # (i, j) -> dma
            lane_nops = {}   # i -> last lane-observation nop
            red3 = {}        # i -> last reduce instr
            ostores = []

            def emit_xload(i):
                xt = xts[i % xbufs]
                dma = nc.sync.dma_start(xt[:], x_d[i * P : (i + 1) * P, :])
                if i >= xbufs:
                    add_dep_helper(
                        dma.ins, i_idxs[i - xbufs].ins, True, "xt WAR"
                    )
                    dma.ins.try_remove_dependency(xls[i - xbufs].ins.name)
                xls.append(dma)
                return dma

            def emit_topk(i):
                xt = xts[i % xbufs]
                s8 = spool.tile([P, 8], F32, name="s8", tag="s8")
                m8 = dveq(nc.vector.max(out=s8[:], in_=xt[:]))
                idx8 = ipool.tile([P, 8], U32, name="idx8", tag="idx8")
                i_idx = dveq(nc.vector.max_index(
                    out=idx8[:], in_max=s8[:], in_values=xt[:]
                ))
                idx8s.append(idx8)
                i_idxs.append(i_idx)
                gws.append(gpool.tile([P, TOPK, e], BF16, name="gw", tag="gw"))

            def emit_gathers(i):
                # gate: the Pool queue waits find_index8_i once, then all 8
                # dep-free gathers flow at Q7 cadence
                gate = poolq(nc.gpsimd.nop())
                add_dep_helper(gate.ins, i_idxs[i].ins, True, "idx ready")
                for j in range(TOPK):
                    dma = nc.gpsimd.indirect_dma_start(
                        out=gws[i][:, j, :],
                        out_offset=None,
                        in_=wt_d[:],
                        in_offset=bass.IndirectOffsetOnAxis(
                            ap=idx8s[i][:, j : j + 1], axis=0
                        ),
                        compute_op=mybir.AluOpType.bypass,
                    )
                    dma.ins.try_remove_dependency(i_idxs[i].ins.name)
                    poolq(dma)
                    gathers[(i, j)] = dma


            def emit_reduce(i):
                # 7 DVE nops each wait one gather's completion; DVE program
                # order accumulates their clocks, so a1 only needs the last
                # gather's wait itself -- one sem wait per instruction.
                # The whole reduce block is pinned AFTER the NEXT tile's
                # find_index8 so the gather latency stays out of the DVE
                # critical loop (max8/find_index must run back-to-back).
                gw = gws[i]
                for j in range(TOPK - 1):
                    vn = nc.vector.nop()
                    strip_sync_deps(vn)
                    add_dep_helper(
                        vn.ins, gathers[(i, j)].ins, True, "gather done"
                    )
                    dveq(vn)
                a1 = nc.vector.tensor_add(
                    gw[:, 0:4, :], gw[:, 0:4, :], gw[:, 4:8, :]
                )
                strip_sync_deps(a1)
                add_dep_helper(
                    a1.ins, gathers[(i, TOPK - 1)].ins, True, "gathers done"
                )
                dveq(a1)
                a2 = nc.vector.tensor_add(
                    gw[:, 0:2, :], gw[:, 0:2, :], gw[:, 2:4, :]
                )
                strip_sync_deps(a2)
                dveq(a2)
                a3 = nc.vector.tensor_add(
                    gw[:, 0, :], gw[:, 0, :], gw[:, 1, :]
                )
                strip_sync_deps(a3)
                dveq(a3)
                red3[i] = a3

            def emit_ostore(i):
                # SWDGE store with bf16 -> f32 cast; gated by a Pool NOP
                # carrying the DVE-reduce dependency
                gate = poolq(nc.gpsimd.nop())
                add_dep_helper(gate.ins, red3[i].ins, True, "reduce done")
                dma = nc.gpsimd.dma_start(
                    out_d[i * P : (i + 1) * P, :], gws[i][:, 0, :]
                )
                strip_sync_deps(dma)
                poolq(dma)
                ostores.append(dma)
                return dma

            for i in range(n_tiles):
                emit_xload(i)
                emit_topk(i)
                if i >= 1:
                    emit_reduce(i - 1)
                emit_gathers(i)
                if i >= 2:
                    emit_ostore(i - 2)
            emit_reduce(n_tiles - 1)
            emit_ostore(n_tiles - 2)
            emit_ostore(n_tiles - 1)

            # Quiesce procs with single-wait SP nops so the kernel-tail
            # drains find their required ticks already observed.
            tail = (
                xls
                + ostores
                + [gathers[(n_tiles - 1, j)] for j in range(TOPK)]
                + [red3[n_tiles - 1]]
                + i_idxs[-1:]
            )
            for tgt in tail:
                n = nc.sync.nop()
                add_dep_helper(n.ins, tgt.ins, True, "tail quiesce")

    return nc


def validate_single_wait(nc):
    """Every instruction may carry at most one semaphore wait."""
    bad = []
    for b in nc.m.functions[0].blocks:
        for ins in b.instructions:
            si = ins.sync_info
            if si is not None and len(si.on_wait) > 1:
                bad.append((ins.name, type(ins).__name__, si.on_wait))
    return bad


def _prep_wt(W: np.ndarray) -> np.ndarray:
    """W [e, q] f32 -> WT [q, e] bf16 contiguous."""
    return np.ascontiguousarray(W.T).astype(ml_dtypes.bfloat16)


_CACHED = {}


def _get_nc():
    if "nc" not in _CACHED:
        _CACHED["nc"] = build_bass()
    return _CACHED["nc"]


def kernel(x: np.ndarray, W: np.ndarray) -> np.ndarray:
    x = np.asarray(x, dtype=np.float32)
    W = np.asarray(W, dtype=np.float32)
    assert x.shape == (B, S, Q) and W.shape == (E, Q)

    nc = _get_nc()
    xf = x.reshape(T_TOTAL, Q)
    WT = _prep_wt(W)
    in_maps = [
        {
            "x": np.ascontiguousarray(xf[c * T_CORE : (c + 1) * T_CORE]),
            "wt": WT,
        }
        for c in range(N_CORES)
    ]
    res = run_bass_kernel_spmd(nc, in_maps, core_ids=list(range(N_CORES)))
    out = np.concatenate([r["out"] for r in res.results], axis=0)
    return np.ascontiguousarray(out.reshape(B, S, E).astype(np.float32))


# revision 32
# speedup vs baseline: 1.2165x; 1.0077x over previous
"""Trainium2 Bass kernel: NKQuantizer2 top-k masking (k=8).

reference:  kh = topk_hot(x, 8)          # [B,S,Q] 0/1 mask, top-8 per token
            out = einsum('bsq,eq->bse', kh, W)

Per token: out[t] = sum_{q in top8(x[t])} W[:, q] -- an 8-way embedding
gather-sum from W.T [Q, E].

Strategy (data-parallel over tokens across 8 cores, W.T bf16 in HBM):
  Per 128-token tile on each core:
    1. DMA x tile [128, 8192] f32 HBM->SBUF on the SP HWDGE ring
    2. DVE Max8 -> top-8 values per token; DVE MaxIndex -> their indices
       (exact, ties -> first occurrence, matching jax.lax.top_k)
    3. 8 INDEPENDENT single-index indirect gathers (bf16, bypass) into a
       wide [128, 8, 512] tile -- no CCE accumulate chain, so the DMAs
       carry no waits and flow at Q7 descriptor-gen cadence (~1.9us).
    4. DVE 3-step tree-reduce (bf16) collapses the 8 rows; a SWDGE store
       casts bf16->f32 to DRAM.

Why no CCE accumulation: a CCE add chain needs each link to wait its
predecessor's DMA *completion* (~4.6us/link serial per tile, and
sem-free accumulation races in the SDMA datapath -- measured).  The
independent-gather design has zero inter-DMA dependencies; the extra
~2.3us/tile of DVE reduce rides on the DVE bottleneck (~17.4 ->
~19.7us/tile) but removes all FIFO head-of-line stalls.

Toolchain constraints handled:
  - at most ONE semaphore wait per instruction: each tile's gather
    block is gated by one Pool NOP carrying the find_index8 wait (the
    Pool queue is serial, so queue order covers the dep-free DMAs);
    the DVE reduce is gated by 7 DVE NOPs each waiting one gather's
    completion (DVE program order accumulates their clocks) plus the
    last gather's wait on the first add itself; the store is gated by
    a Pool NOP carrying the reduce dependency.
  - both the Pool queue and the DVE stream are nosync-chained in
    emission order so the scheduler cannot reorder them -- the DVE
    order max8/find_index8 back-to-back with the previous tile's
    reduce slotted after is what keeps the gather latency out of the
    DVE critical loop (~19.4us/tile measured vs 17.4 pure top-k).
"""

import numpy as np
import ml_dtypes

import concourse.bass as bass
import concourse.mybir as mybir
import concourse.tile as tile
from concourse.bass_utils import run_bass_kernel_spmd
from concourse.tile_rust import add_dep_helper

B, S, Q, E, TOPK = 4, 2048, 8192, 512, 8
N_CORES = 8
P = 128
T_TOTAL = B * S                 # 8192 tokens
T_CORE = T_TOTAL // N_CORES     # 1024 tokens per core

F32 = mybir.dt.float32
BF16 = mybir.dt.bfloat16
U32 = mybir.dt.uint32


def build_bass(t_core=T_CORE, q=Q, e=E):
    """Build the per-core Bass program (SPMD: same program on all cores)."""
    n_tiles = t_core // P
    assert n_tiles == 8
    xbufs = 4

    nc = bass.Bass(trn_type="TRN2", target_bir_lowering=False)
    x_d = nc.dram_tensor("x", [t_core, q], F32, kind="ExternalInput")
    wt_d = nc.dram_tensor("wt", [q, e], BF16, kind="ExternalInput")
    out_d = nc.dram_tensor("out", [t_core, e], F32, kind="ExternalOutput")

    pool_q = []  # ALL Pool-queue instructions (DMAs + NOPs) in queue order
    dve_q = []   # ALL DVE instructions in intended execution order

    def strip_sync_deps(bi):
        try:
            names = list(bi.ins.sync_dependency_names())
        except TypeError:
            names = list(bi.ins.sync_dependency_names)
        for n in names:
            bi.ins.try_remove_dependency(n)

    def poolq(bi):
        """nosync-chain every Pool instruction in emission order."""
        if pool_q:
            add_dep_helper(bi.ins, pool_q[-1].ins, False, "pool queue order")
        pool_q.append(bi)
        return bi

    def dveq(bi):
        """nosync-chain every DVE instruction in emission order."""
        if dve_q:
            add_dep_helper(bi.ins, dve_q[-1].ins, False, "dve queue order")
        dve_q.append(bi)
        return bi

    with tile.TileContext(nc) as tc:
        with (
            tc.tile_pool(name="xpool", bufs=xbufs) as xpool,
            tc.tile_pool(name="spool", bufs=n_tiles) as spool,
            tc.tile_pool(name="ipool", bufs=n_tiles) as ipool,
            tc.tile_pool(name="gpool", bufs=n_tiles) as gpool,
        ):
            xts = [xpool.tile([P, q], F32, name="xt", tag="xt") for _ in range(xbufs)]
            idx8s, gws, i_idxs, xls = [], [], [], []
            gathers = {}     # (i, j) -> dma
            red3 = {}        # i -> last reduce instr
            ostores = []

            def emit_xload(i):
                xt = xts[i % xbufs]
                dma = nc.sync.dma_start(xt[:], x_d[i * P : (i + 1) * P, :])
                if i >= xbufs:
                    add_dep_helper(
                        dma.ins, i_idxs[i - xbufs].ins, True, "xt WAR"
                    )
                    dma.ins.try_remove_dependency(xls[i - xbufs].ins.name)
                xls.append(dma)
                return dma

            def emit_topk(i):
                xt = xts[i % xbufs]
                s8 = spool.tile([P, 8], F32, name="s8", tag="s8")
                m8 = dveq(nc.vector.max(out=s8[:], in_=xt[:]))
                idx8 = ipool.tile([P, 8], U32, name="idx8", tag="idx8")
                i_idx = dveq(nc.vector.max_index(
                    out=idx8[:], in_max=s8[:], in_values=xt[:]
                ))
                idx8s.append(idx8)
                i_idxs.append(i_idx)
                gws.append(gpool.tile([P, TOPK, e], BF16, name="gw", tag="gw"))

            def emit_gathers(i):
                # gate: the Pool queue waits find_index8_i once, then all 8
                # dep-free gathers flow at Q7 cadence
                gate = poolq(nc.gpsimd.nop())
                add_dep_helper(gate.ins, i_idxs[i].ins, True, "idx ready")
                for j in range(TOPK):
                    dma = nc.gpsimd.indirect_dma_start(
                        out=gws[i][:, j, :],
                        out_offset=None,
                        in_=wt_d[:],
                        in_offset=bass.IndirectOffsetOnAxis(
                            ap=idx8s[i][:, j : j + 1], axis=0
                        ),
                        compute_op=mybir.AluOpType.bypass,
                    )
                    dma.ins.try_remove_dependency(i_idxs[i].ins.name)
                    poolq(dma)
                    gathers[(i, j)] = dma


            def emit_reduce(i):
                # 7 DVE nops each wait one gather's completion; DVE program
                # order accumulates their clocks, so a1 only needs the last
                # gather's wait itself -- one sem wait per instruction.
                # The whole reduce block is pinned AFTER the NEXT tile's
                # find_index8 so the gather latency stays out of the DVE
                # critical loop (max8/find_index must run back-to-back).
                gw = gws[i]
                for j in range(TOPK - 1):
                    vn = nc.vector.nop()
                    strip_sync_deps(vn)
                    add_dep_helper(
                        vn.ins, gathers[(i, j)].ins, True, "gather done"
                    )
                    dveq(vn)
                a1 = nc.vector.tensor_add(
                    gw[:, 0:4, :], gw[:, 0:4, :], gw[:, 4:8, :]
                )
                strip_sync_deps(a1)
                add_dep_helper(
                    a1.ins, gathers[(i, TOPK - 1)].ins, True, "gathers done"
                )
                dveq(a1)
                a2 = nc.vector.tensor_add(
                    gw[:, 0:2, :], gw[:, 0:2, :], gw[:, 2:4, :]
                )
                strip_sync_deps(a2)
                dveq(a2)
                a3 = nc.vector.tensor_add(
                    gw[:, 0, :], gw[:, 0, :], gw[:, 1, :]
                )
                strip_sync_deps(a3)
                dveq(a3)
                red3[i] = a3

            def emit_ostore(i):
                # SWDGE store with bf16 -> f32 cast; gated by a Pool NOP
                # carrying the DVE-reduce dependency
                gate = poolq(nc.gpsimd.nop())
                add_dep_helper(gate.ins, red3[i].ins, True, "reduce done")
                dma = nc.gpsimd.dma_start(
                    out_d[i * P : (i + 1) * P, :], gws[i][:, 0, :]
                )
                strip_sync_deps(dma)
                poolq(dma)
                ostores.append(dma)
                return dma

            for i in range(n_tiles):
                emit_xload(i)
                emit_topk(i)
                if i >= 1:
                    emit_reduce(i - 1)
                emit_gathers(i)
                if i >= 2:
                    emit_ostore(i - 2)
            emit_reduce(n_tiles - 1)
            emit_ostore(n_tiles - 2)
            emit_ostore(n_tiles - 1)

            # Quiesce procs with single-wait SP nops so the kernel-tail
            # drains find their required ticks already observed.
            tail = (
                xls
                + ostores
                + [gathers[(n_tiles - 1, j)] for j in range(TOPK)]
                + [red3[n_tiles - 1]]
                + i_idxs[-1:]
            )
            for tgt in tail:
                n = nc.sync.nop()
                add_dep_helper(n.ins, tgt.ins, True, "tail quiesce")

    return nc


def validate_single_wait(nc):
    """Every instruction may carry at most one semaphore wait."""
    bad = []
    for b in nc.m.functions[0].blocks:
        for ins in b.instructions:
            si = ins.sync_info
            if si is not None and len(si.on_wait) > 1:
                bad.append((ins.name, type(ins).__name__, si.on_wait))
    return bad


def _prep_wt(W: np.ndarray) -> np.ndarray:
    """W [e, q] f32 -> WT [q, e] bf16 contiguous."""
    return np.ascontiguousarray(W.T).astype(ml_dtypes.bfloat16)


_CACHED = {}


def _get_nc():
    if "nc" not in _CACHED:
        _CACHED["nc"] = build_bass()
    return _CACHED["nc"]


def kernel(x: np.ndarray, W: np.ndarray) -> np.ndarray:
    x = np.asarray(x, dtype=np.float32)
    W = np.asarray(W, dtype=np.float32)
    assert x.shape == (B, S, Q) and W.shape == (E, Q)

    nc = _get_nc()
    xf = x.reshape(T_TOTAL, Q)
    WT = _prep_wt(W)
    in_maps = [
        {
            "x": np.ascontiguousarray(xf[c * T_CORE : (c + 1) * T_CORE]),
            "wt": WT,
        }
        for c in range(N_CORES)
    ]
    res = run_bass_kernel_spmd(nc, in_maps, core_ids=list(range(N_CORES)))
    out = np.concatenate([r["out"] for r in res.results], axis=0)
    return np.ascontiguousarray(out.reshape(B, S, E).astype(np.float32))
